# revision 22
# baseline (speedup 1.0000x reference)
import sys
import numpy as np
from contextlib import ExitStack

sys.path.insert(0, "/opt/trn_rl_repo")

import concourse.bass as bass
import concourse.tile as tile
from concourse import bacc, mybir
from concourse import bass_utils

FP32 = mybir.dt.float32
FP32R = mybir.dt.float32r
FP16 = mybir.dt.float16
I16 = mybir.dt.int16
AF = mybir.ActivationFunctionType
ALU = mybir.AluOpType

B, CIN, H, W = 8, 256, 96, 96
R = 32
COUT = 256
NPIX = H * W
NCHUNK = 5
NT = 512
W_STRIP = 1024
NS = NPIX // W_STRIP
TPS = W_STRIP // NT
N_CORES = 8


def _build_assignment():
    starts = [5 * k for k in range(7)]
    offs = np.cumsum([0] + [32 - s for s in starts])
    assert offs[-1] <= 128
    rho = np.zeros(128, np.int64)
    for k, s in enumerate(starts):
        for q in range(32 - s):
            rho[offs[k] + q] = s + q
    tri_i, tri_j = np.triu_indices(R)
    torig_of = {(int(a), int(b)): t for t, (a, b) in enumerate(zip(tri_i, tri_j))}
    i_map = np.zeros((NCHUNK, 128), np.int64)
    torig = -np.ones((NCHUNK, 128), np.int64)
    placed = 0
    for c in range(NCHUNK):
        for k, s in enumerate(starts):
            blk = s + c
            i = blk if blk <= 31 else 0
            for q in range(32 - s):
                j = s + q
                p = offs[k] + q
                i_map[c, p] = i
                if blk <= 31 and j >= blk:
                    torig[c, p] = torig_of[(blk, j)]
                    placed += 1
        i_map[c, offs[-1]:] = 0
    assert placed == len(tri_i), f"assignment failed: {placed}/{len(tri_i)}"
    return rho, i_map, torig


_RHO, _IMAP, _TORIG = _build_assignment()


def _pack_weights(w_reduce, w_recover):
    w4j = w_reduce[_RHO, :].T.astype(np.float32)
    w4j_packed = np.ascontiguousarray(
        np.concatenate([w4j[:128, :], w4j[128:, :]], axis=1))
    wr = np.zeros((NCHUNK, 128, COUT), np.float32)
    for c in range(NCHUNK):
        for p in range(128):
            t = _TORIG[c, p]
            if t >= 0:
                wr[c, p, :] = w_recover[:, t]
    blocks = []
    for c in range(NCHUNK):
        for m in range(2):
            blocks.append(wr[c, :, m * 128:(m + 1) * 128])
    wrec_packed = np.concatenate(blocks, axis=1).astype(np.float32)
    sel = np.zeros((R, NCHUNK * 128), np.float32)
    for c in range(NCHUNK):
        sel[_IMAP[c], c * 128 + np.arange(128)] = 1.0
    return w4j_packed, wrec_packed, sel


_NC_CACHE = {}


def _build_program(reps=1, stage=5, loop=0):
    key = (reps, stage, loop)
    if key in _NC_CACHE:
        return _NC_CACHE[key]
    nc = bacc.Bacc("TRN2", debug=False, num_devices=N_CORES)
    x_d = nc.dram_tensor("x", [CIN, NPIX], FP32, kind="ExternalInput")
    w4j_d = nc.dram_tensor("w4j", [128, 256], FP32, kind="ExternalInput")
    wrec_d = nc.dram_tensor("wrec", [128, 1280], FP32, kind="ExternalInput")
    sel_d = nc.dram_tensor("sel", [R, NCHUNK * 128], FP32, kind="ExternalInput")
    y_d = nc.dram_tensor("y", [COUT, NPIX], FP16, kind="ExternalOutput")

    with tile.TileContext(nc) as tc, ExitStack() as ctx:
        wp = ctx.enter_context(tc.tile_pool(name="wp", bufs=1))
        xp = ctx.enter_context(tc.tile_pool(name="xp", bufs=2))
        o4p = ctx.enter_context(tc.tile_pool(name="o4p", bufs=2))
        upp = ctx.enter_context(tc.tile_pool(name="upp", bufs=2))
        y16p = ctx.enter_context(tc.tile_pool(name="y16p", bufs=2))
        epp = ctx.enter_context(tc.tile_pool(name="epp", bufs=2))
        ps_o4 = ctx.enter_context(tc.tile_pool(name="ps_o4", bufs=2, space="PSUM"))
        ps_ci = ctx.enter_context(tc.tile_pool(name="ps_ci", bufs=2, space="PSUM"))
        ps_y = ctx.enter_context(tc.tile_pool(name="ps_y", bufs=2, space="PSUM"))

        w4j_sb = wp.tile([128, 256], FP32R)
        nc.sync.dma_start(w4j_sb[:], w4j_d.ap().bitcast(FP32R))
        wrec_sb = wp.tile([128, 1280], FP32R)
        nc.sync.dma_start(wrec_sb[:], wrec_d.ap().bitcast(FP32R))
        sel_sb = wp.tile([R, NCHUNK * 128], FP32R)
        nc.sync.dma_start(sel_sb[:], sel_d.ap().bitcast(FP32R))
        eps_sb = wp.tile([128, 1], FP32)
        nc.gpsimd.memset(eps_sb[:], 1e-6)
        msk_sb = wp.tile([128, 1], I16)
        nc.gpsimd.memset(msk_sb[:], -32768)
        amsk_sb = wp.tile([128, 1], I16)
        nc.gpsimd.memset(amsk_sb[:], 32767)

        loop_cm = tc.For_i(0, loop, 1) if loop else None
        if loop_cm is not None:
            loop_cm.__enter__()
        for rep in range(reps):
            for s in range(NS):
                s0 = s * W_STRIP
                x_sb = xp.tile([128, 2 * W_STRIP], FP32R,
                               name=f"x_{rep}_{s}", tag="x")
                nc.sync.dma_start(x_sb[:, 0:W_STRIP],
                                  x_d.ap()[0:128, s0:s0 + W_STRIP]
                                  .bitcast(FP32R))
                nc.sync.dma_start(x_sb[:, W_STRIP:],
                                  x_d.ap()[128:256, s0:s0 + W_STRIP]
                                  .bitcast(FP32R))
                out4 = o4p.tile([128, W_STRIP], FP32R, name=f"o4_{rep}_{s}",
                                tag="o4")
                for tt in range(TPS):
                    c0 = tt * NT
                    o4_ps = ps_o4.tile([128, NT], FP32,
                                       name=f"o4ps_{rep}_{s}_{tt}", tag="o4ps")
                    nc.tensor.matmul(o4_ps[:], w4j_sb[:, 0:128],
                                     x_sb[:, c0:c0 + NT],
                                     start=True, stop=False)
                    nc.tensor.matmul(o4_ps[:], w4j_sb[:, 128:256],
                                     x_sb[:, W_STRIP + c0:W_STRIP + c0 + NT],
                                     start=False, stop=True)
                    nc.scalar.copy(out4[:, c0:c0 + NT], o4_ps[:])

                if stage < 3:
                    continue
                up = [upp.tile([128, W_STRIP], FP32R, name=f"up{c}_{rep}_{s}",
                               tag=f"up{c}") for c in range(NCHUNK)]
                for c in range(NCHUNK):
                    ci_ps = ps_ci.tile([128, W_STRIP], FP32,
                                       name=f"ci{c}_{rep}_{s}", tag="ci")
                    for tt in range(TPS):
                        nc.tensor.matmul(
                            ci_ps[:, tt * NT:(tt + 1) * NT],
                            sel_sb[:, c * 128:(c + 1) * 128],
                            out4[0:R, tt * NT:(tt + 1) * NT],
                            start=True, stop=True)
                    nc.vector.tensor_mul(up[c][:], ci_ps[:], out4[:])

                if stage < 4:
                    continue
                y16 = [y16p.tile([128, W_STRIP], FP16,
                                 name=f"y16_{m}_{rep}_{s}", tag=f"y16_{m}")
                       for m in range(2)]
                for tt in range(TPS):
                    for m in range(2):
                        yp = ps_y.tile([128, NT], FP32,
                                       name=f"yp{m}_{rep}_{s}_{tt}", tag="y")
                        for c in range(NCHUNK):
                            wcol = (c * 2 + m) * 128
                            nc.tensor.matmul(
                                yp[:], wrec_sb[:, wcol:wcol + 128],
                                up[c][:, tt * NT:(tt + 1) * NT],
                                start=(c == 0), stop=(c == NCHUNK - 1))
                        nc.scalar.copy(y16[m][:, tt * NT:(tt + 1) * NT], yp[:])

                if stage < 5:
                    continue
                for m in range(2):
                    ysl = y16[m][:]
                    a16 = epp.tile([128, W_STRIP], I16,
                                   name=f"a16_{m}_{rep}_{s}", tag="a")
                    nc.vector.tensor_scalar(a16[:], ysl.bitcast(I16),
                                            amsk_sb[:], None, ALU.bitwise_and)
                    s16 = epp.tile([128, W_STRIP], FP16,
                                   name=f"s16_{m}_{rep}_{s}", tag="s")
                    nc.scalar.activation(s16[:], a16[:].bitcast(FP16),
                                         AF.Sqrt, bias=eps_sb[:])
                    t16 = epp.tile([128, W_STRIP], FP16,
                                   name=f"t16_{m}_{rep}_{s}", tag="t")
                    nc.gpsimd.tensor_scalar(t16[:], ysl, 0.0, None, ALU.is_ge)
                    g16 = epp.tile([128, W_STRIP], FP16,
                                   name=f"g16_{m}_{rep}_{s}", tag="g")
                    nc.gpsimd.tensor_scalar(g16[:], t16[:], 2.0, -1.0,
                                            ALU.mult, ALU.add)
                    o16 = epp.tile([128, W_STRIP], FP16,
                                   name=f"o16_{m}_{rep}_{s}", tag="o")
                    nc.gpsimd.tensor_tensor(o16[:], s16[:], g16[:], ALU.mult)
                    nc.sync.dma_start(
                        y_d.ap()[m * 128:(m + 1) * 128, s0:s0 + W_STRIP],
                        o16[:])
        if loop_cm is not None:
            loop_cm.__exit__(None, None, None)
    nc.compile()
    _NC_CACHE[key] = nc
    return nc


def _in_maps(inputs):
    x = np.asarray(inputs["x"], np.float32)
    w4j, wrec, sel = _pack_weights(
        np.asarray(inputs["w_reduce"], np.float32),
        np.asarray(inputs["w_recover"], np.float32))
    return [{"x": np.ascontiguousarray(x[b].reshape(CIN, NPIX)),
             "w4j": w4j, "wrec": wrec, "sel": sel} for b in range(N_CORES)]


def kernel(x, w_reduce, w_recover):
    nc = _build_program()
    in_maps = _in_maps({"x": x, "w_reduce": w_reduce, "w_recover": w_recover})
    res = bass_utils.run_bass_kernel_spmd(nc, in_maps,
                                          core_ids=list(range(N_CORES)))
    out = np.stack([res.results[b]["y"].reshape(COUT, H, W)
                    for b in range(N_CORES)])
    return out.astype(np.float32)
